# revision 9
# baseline (speedup 1.0000x reference)
"""Bidirectional masked softmax geometric-mean kernel for Trainium2 (8 cores).

Problem: for each batch b (8 total):
  mask[i,j] = (i < L1_b) & (j < L2_b)
  logits    = where(mask, sim/TAU, -1e30)
  out       = where(mask, sqrt(EPS + softmax_row(logits) * softmax_col(logits)), 0)

Sharding: data-parallel over batch: core c handles slab c ([2048,2048] f32).

Math: with a fixed global stabilizer M (valid upper bound on logits),
  row_sm * col_sm = E^2 / (R_i * C_j),  E = exp(x/TAU - M),
  R_i = sum_j E (masked), C_j = sum_i E (masked)
so no per-row/col max pass is needed; exp underflow is benign because the
EPS floor dominates anything below 1e-8.

Device pipeline per 128-row tile (16 tiles):
  pass1: DVE add col-mask bias -> ACT exp (row bias; accum_out = row sums)
         -> PE per-128-col-block col-sum matmuls accumulated in PSUM [128,16]
  mid:   fixup+reciprocal of R and C in [128,16] layouts (128-lane),
         DRAM-bounce transpose of 1/C to a [1,2048] row, broadcast to [128,2048]
  pass2: ACT square -> DVE mul by 1/C -> ACT sqrt (scale=1/R_i, bias=EPS*rmask_i)
         -> GPSIMD mul by col mask -> DMA out
"""

import numpy as np
from contextlib import ExitStack

import concourse.bass as bass
import concourse.mybir as mybir
import concourse.tile as tile
from concourse.bass_utils import run_bass_kernel_spmd

B = 8
L = 2048
P = 128
NT = L // P  # 16 row tiles / col blocks
TAU = 0.5
EPS = 1e-8
MSTAB = 24.0  # global stabilizer in logit (x/TAU) units; logits are within ~±11
NEGB = 30000.0  # additive -inf substitute (exp underflows to exactly 0)
F32 = mybir.dt.float32

_CACHE = {}


HALF = 1024  # lengths are >= 1024, so columns [0, 1024) are always valid


def _body(ctx, tc, x, cneg, cmask, rbiasT, sbiasT, rfixT, cfixT, cdram, y):
    nc = tc.nc
    Exp = mybir.ActivationFunctionType.Exp
    Sqrt = mybir.ActivationFunctionType.Sqrt
    Square = mybir.ActivationFunctionType.Square

    singles = ctx.enter_context(tc.tile_pool(name="singles", bufs=1))
    xpool = ctx.enter_context(tc.tile_pool(name="xp", bufs=4))
    epool = ctx.enter_context(tc.tile_pool(name="ep", bufs=NT))
    pspool = ctx.enter_context(tc.tile_pool(name="ps", bufs=2, space="PSUM"))

    # --- constants / per-row vectors (right-half masks only) ---
    cneg_h = singles.tile([P, L - HALF], F32, tag="cneg_h")
    nc.sync.dma_start(out=cneg_h, in_=cneg[0:1, HALF:].to_broadcast([P, L - HALF]))
    cmask_h = singles.tile([P, L - HALF], F32, tag="cmask_h")
    nc.sync.dma_start(out=cmask_h, in_=cmask[0:1, HALF:].to_broadcast([P, L - HALF]))

    rbias_sb = singles.tile([P, NT], F32, tag="rbias")
    nc.sync.dma_start(out=rbias_sb, in_=rbiasT[:, :])
    sbias_sb = singles.tile([P, NT], F32, tag="sbias")
    nc.sync.dma_start(out=sbias_sb, in_=sbiasT[:, :])
    rfix_sb = singles.tile([P, NT], F32, tag="rfix")
    nc.sync.dma_start(out=rfix_sb, in_=rfixT[:, :])
    cfix_sb = singles.tile([P, NT], F32, tag="cfix")
    nc.sync.dma_start(out=cfix_sb, in_=cfixT[:, :])

    ones_sb = singles.tile([P, 1], F32, tag="ones")
    nc.vector.memset(ones_sb, 1.0)

    Rsum = singles.tile([P, NT], F32, tag="Rsum")
    invR = singles.tile([P, NT], F32, tag="invR")
    Csum = singles.tile([P, NT], F32, tag="Csum")
    invCT = singles.tile([P, NT], F32, tag="invCT")
    invC_b = singles.tile([P, L], F32, tag="invC_b")
    nc.vector.memset(Csum, 0.0)

    E_tiles = [epool.tile([P, L], F32, tag="E", name=f"E{t}") for t in range(NT)]

    # --- pass 1: E = exp(2*(x + cneg) + rbias); R rowsums; C colsums;
    #     then square E in place (pass 2 only needs E^2) ---
    for t in range(NT):
        xt = xpool.tile([P, L], F32, tag="xt")
        nc.sync.dma_start(out=xt, in_=x[t * P : (t + 1) * P, :])
        nc.vector.tensor_add(xt[:, HALF:], xt[:, HALF:], cneg_h)
        Et = E_tiles[t]
        nc.scalar.activation(
            Et,
            xt,
            Exp,
            bias=rbias_sb[:, t : t + 1],
            scale=2.0,
            accum_out=Rsum[:, t : t + 1],
        )
        # col-block sums of this tile: Cp_t[:, c] = Et[:, cP:(c+1)P].T @ ones
        Cp = pspool.tile([P, NT], F32, tag="Cp")
        for c in range(NT):
            nc.tensor.matmul(
                Cp[:, c : c + 1],
                Et[:, c * P : (c + 1) * P],
                ones_sb,
                start=True,
                stop=True,
            )
        nc.vector.tensor_add(Csum, Csum, Cp)
        nc.scalar.activation(Et, Et, Square)

    # --- mid: reciprocals ---
    nc.vector.tensor_add(Rsum, Rsum, rfix_sb)
    nc.vector.reciprocal(invR, Rsum)

    nc.vector.tensor_add(Csum, Csum, cfix_sb)
    nc.vector.reciprocal(invCT, Csum)

    # invCT[p, c] holds 1/C_j for j = c*128 + p. Bounce through DRAM to get a
    # j-ordered row, then broadcast-read it across all 128 partitions.
    nc.sync.dma_start(out=cdram[:, :].rearrange("c p -> p c"), in_=invCT[:, :])
    cd0 = cdram[:, :]
    flat_bcast = bass.AP(tensor=cd0.tensor, offset=cd0.offset, ap=[[0, P], [1, L]])
    nc.sync.dma_start(out=invC_b, in_=flat_bcast)

    # --- pass 2: out = cmask * sqrt(E^2 * invC * invR + EPS*rmask) ---
    for t in range(NT):
        Et = E_tiles[t]  # holds E^2
        Pt = xpool.tile([P, L], F32, tag="xt")
        nc.vector.tensor_mul(Pt, Et, invC_b)
        nc.scalar.activation(
            Et, Pt, Sqrt, bias=sbias_sb[:, t : t + 1], scale=invR[:, t : t + 1]
        )
        nc.gpsimd.tensor_mul(Et[:, HALF:], Et[:, HALF:], cmask_h)
        nc.sync.dma_start(out=y[t * P : (t + 1) * P, :], in_=Et)


def _split_multi_waits(nc):
    """This walrus build's CoreV3 setupSyncWait rejects ANY instruction
    carrying more than one semaphore wait ("Too many sync wait commands");
    the ISA Events header has a single wait slot. Hoist extra waits onto
    preceding same-engine NoOps (sequential ge-waits on monotonic semaphores
    are equivalent to a combined wait). Apply only for the HW path — the
    synthetic NoOps lack the sim's sem bookkeeping and break CoreSim."""
    n = 0
    for fn in nc.m.functions:
        for bb in fn.blocks:
            out = []
            changed = False
            for inst in bb.instructions:
                si = inst.sync_info
                waits = list(si.on_wait) if (si and si.on_wait) else []
                if len(waits) > 1:
                    for w in waits[:-1]:
                        n += 1
                        out.append(
                            mybir.InstNoOp(
                                name=f"antsplitwait-{n}",
                                engine=inst.engine,
                                sync_info=mybir.SyncInfo(on_wait=[w], on_update=[]),
                            )
                        )
                    si.on_wait = waits[-1:]
                    changed = True
                out.append(inst)
            if changed:
                bb.instructions = out
    return nc


def build_nc(split_waits=True):
    nc = bass.Bass()
    x = nc.dram_tensor("x", [L, L], F32, kind="ExternalInput")
    cneg = nc.dram_tensor("cneg", [1, L], F32, kind="ExternalInput")
    cmask = nc.dram_tensor("cmask", [1, L], F32, kind="ExternalInput")
    rbiasT = nc.dram_tensor("rbiasT", [P, NT], F32, kind="ExternalInput")
    sbiasT = nc.dram_tensor("sbiasT", [P, NT], F32, kind="ExternalInput")
    rfixT = nc.dram_tensor("rfixT", [P, NT], F32, kind="ExternalInput")
    cfixT = nc.dram_tensor("cfixT", [P, NT], F32, kind="ExternalInput")
    cdram = nc.dram_tensor("cscratch", [NT, P], F32, kind="Internal")
    y = nc.dram_tensor("y", [L, L], F32, kind="ExternalOutput")

    with tile.TileContext(nc) as tc, ExitStack() as ctx:
        _body(ctx, tc, x, cneg, cmask, rbiasT, sbiasT, rfixT, cfixT, cdram, y)
    if split_waits:
        _split_multi_waits(nc)
    return nc


def get_nc():
    if "nc" not in _CACHE:
        _CACHE["nc"] = build_nc()
    return _CACHE["nc"]


def make_in_maps(sim_matrix, lengths):
    sim_matrix = np.ascontiguousarray(np.asarray(sim_matrix, dtype=np.float32))
    lengths = np.asarray(lengths, dtype=np.int32)
    idx = np.arange(L)
    in_maps = []
    for c in range(sim_matrix.shape[0]):
        l1, l2 = int(lengths[c, 0]), int(lengths[c, 1])
        rv = idx < l1  # row valid
        cv = idx < l2  # col valid

        def tcol(vals):  # [2048] -> [128, 16] with element i at [i%128, i//128]
            return np.ascontiguousarray(
                np.asarray(vals, dtype=np.float32).reshape(NT, P).T
            )

        in_maps.append(
            {
                "x": sim_matrix[c],
                "cneg": np.where(cv, 0.0, -NEGB / 2).astype(np.float32)[None, :],
                "cmask": cv.astype(np.float32)[None, :],
                "rbiasT": tcol(np.where(rv, -MSTAB, -MSTAB - NEGB)),
                "sbiasT": tcol(np.where(rv, EPS, 0.0)),
                "rfixT": tcol(np.where(rv, 0.0, 1.0)),
                "cfixT": tcol(np.where(cv, 0.0, 1.0)),
            }
        )
    return in_maps


def run(sim_matrix, lengths, trace=False):
    nc = get_nc()
    in_maps = make_in_maps(sim_matrix, lengths)
    res = run_bass_kernel_spmd(nc, in_maps, list(range(B)), trace=trace)
    out = np.stack([res.results[c]["y"] for c in range(B)], axis=0)
    return out, res


def kernel(sim_matrix, lengths):
    out, _ = run(sim_matrix, lengths, trace=False)
    return out


# revision 12
# speedup vs baseline: 1.5047x; 1.5047x over previous
"""Bidirectional masked softmax geometric-mean kernel for Trainium2 (8 cores).

Problem: for each batch b (8 total):
  mask[i,j] = (i < L1_b) & (j < L2_b)
  logits    = where(mask, sim/TAU, -1e30)
  out       = where(mask, sqrt(EPS + softmax_row(logits) * softmax_col(logits)), 0)

Sharding: data-parallel over batch: core c handles slab c ([2048,2048] f32).

Math: with a fixed global stabilizer M (valid upper bound on logits),
  row_sm * col_sm = E^2 / (R_i * C_j),  E = exp(x/TAU - M),
  R_i = sum_j E (masked), C_j = sum_i E (masked)
so no per-row/col max pass is needed; exp underflow is benign because the
EPS floor dominates anything below 1e-8.

Device pipeline per 128-row tile (16 tiles):
  pass1: DVE add col-mask bias -> ACT exp (row bias; accum_out = row sums)
         -> PE per-128-col-block col-sum matmuls accumulated in PSUM [128,16]
  mid:   fixup+reciprocal of R and C in [128,16] layouts (128-lane),
         DRAM-bounce transpose of 1/C to a [1,2048] row, broadcast to [128,2048]
  pass2: ACT square -> DVE mul by 1/C -> ACT sqrt (scale=1/R_i, bias=EPS*rmask_i)
         -> GPSIMD mul by col mask -> DMA out
"""

import numpy as np
from contextlib import ExitStack

import concourse.bass as bass
import concourse.mybir as mybir
import concourse.tile as tile
from concourse.bass_utils import run_bass_kernel_spmd

B = 8
L = 2048
P = 128
NT = L // P  # 16 row tiles / col blocks
TAU = 0.5
EPS = 1e-8
MSTAB = 24.0  # global stabilizer in logit (x/TAU) units; logits are within ~±11
NEGB = 30000.0  # additive -inf substitute (exp underflows to exactly 0)
F32 = mybir.dt.float32

_CACHE = {}


HALF = 1024  # lengths are >= 1024, so columns [0, 1024) are always valid
CH = 512  # matmul free-dim chunk (PSUM bank limit)
NCH = L // CH  # 4 colsum accumulation chains


def _body(ctx, tc, x, cneg, cmask, auxT, cdram, crdram, y):
    nc = tc.nc
    Exp = mybir.ActivationFunctionType.Exp
    Sqrt = mybir.ActivationFunctionType.Sqrt
    Square = mybir.ActivationFunctionType.Square

    singles = ctx.enter_context(tc.tile_pool(name="singles", bufs=1))
    xpool = ctx.enter_context(tc.tile_pool(name="xp", bufs=4))
    epool = ctx.enter_context(tc.tile_pool(name="ep", bufs=NT))
    pspool = ctx.enter_context(tc.tile_pool(name="ps", bufs=NCH, space="PSUM"))

    # --- constants / per-row vectors (right-half masks only) ---
    cneg_h = singles.tile([P, L - HALF], F32, tag="cneg_h")
    nc.sync.dma_start(out=cneg_h, in_=cneg[0:1, HALF:].to_broadcast([P, L - HALF]))
    cmask_h = singles.tile([P, L - HALF], F32, tag="cmask_h")
    nc.sync.dma_start(out=cmask_h, in_=cmask[0:1, HALF:].to_broadcast([P, L - HALF]))

    aux_sb = singles.tile([P, 4 * NT], F32, tag="aux")
    nc.sync.dma_start(out=aux_sb, in_=auxT[:, :])
    rbias_sb = aux_sb[:, 0:NT]
    sbias_sb = aux_sb[:, NT : 2 * NT]
    rfix_sb = aux_sb[:, 2 * NT : 3 * NT]
    cfix_sb = aux_sb[:, 3 * NT : 4 * NT]

    ones_sb = singles.tile([P, 1], F32, tag="ones")
    nc.vector.memset(ones_sb, 1.0)

    Rsum = singles.tile([P, NT], F32, tag="Rsum")
    invR = singles.tile([P, NT], F32, tag="invR")
    Crow = singles.tile([1, L], F32, tag="Crow")
    CT = singles.tile([P, NT], F32, tag="CT")
    invCT = singles.tile([P, NT], F32, tag="invCT")
    invC_b = singles.tile([P, L], F32, tag="invC_b")

    E_tiles = [epool.tile([P, L], F32, tag="E", name=f"E{t}") for t in range(NT)]
    # 4 colsum accumulators [1, 512], one PSUM bank each; chain over t per chunk
    Cps = [pspool.tile([1, CH], F32, tag="Cps", name=f"Cps{c}") for c in range(NCH)]

    # --- pass 1: E = exp(2*(x + cneg) + rbias); R rowsums; C colsums;
    #     then square E in place (pass 2 only needs E^2) ---
    for t in range(NT):
        xt = xpool.tile([P, L], F32, tag="xt")
        nc.sync.dma_start(out=xt, in_=x[t * P : (t + 1) * P, :])
        nc.vector.tensor_add(xt[:, HALF:], xt[:, HALF:], cneg_h)
        Et = E_tiles[t]
        nc.scalar.activation(
            Et,
            xt,
            Exp,
            bias=rbias_sb[:, t : t + 1],
            scale=2.0,
            accum_out=Rsum[:, t : t + 1],
        )
        # colsum chains: Cps[c][0, :] += ones.T @ Et[:, chunk c]  (ones stationary)
        for c in range(NCH):
            nc.tensor.matmul(
                Cps[c][:, :],
                ones_sb,
                Et[:, c * CH : (c + 1) * CH],
                start=(t == 0),
                stop=(t == NT - 1),
            )
        nc.scalar.activation(Et, Et, Square)

    # --- mid: reciprocals ---
    nc.vector.tensor_add(Rsum, Rsum, rfix_sb)
    nc.vector.reciprocal(invR, Rsum)

    # psum [1,512]x4 -> Crow [1, 2048] -> (dram bounce) -> CT [128,16] with
    # CT[p, c] = C_j for j = c*128 + p; fix up masked cols; reciprocal;
    # bounce back to a j-ordered row and broadcast to all 128 partitions.
    for c in range(NCH):
        nc.scalar.copy(Crow[0:1, c * CH : (c + 1) * CH], Cps[c][:, :])
    nc.sync.dma_start(out=crdram[0:1, :], in_=Crow)
    cr0 = crdram[:, :]
    crT = bass.AP(tensor=cr0.tensor, offset=cr0.offset, ap=[[1, P], [P, NT]])
    nc.sync.dma_start(out=CT, in_=crT)
    nc.vector.tensor_add(CT, CT, cfix_sb)
    nc.vector.reciprocal(invCT, CT)
    nc.sync.dma_start(out=cdram[:, :].rearrange("c p -> p c"), in_=invCT[:, :])
    cd0 = cdram[:, :]
    flat_bcast = bass.AP(tensor=cd0.tensor, offset=cd0.offset, ap=[[0, P], [1, L]])
    nc.sync.dma_start(out=invC_b, in_=flat_bcast)

    # --- pass 2: out = cmask * sqrt(E^2 * invC * invR + EPS*rmask) ---
    for t in range(NT):
        Et = E_tiles[t]  # holds E^2
        Pt = xpool.tile([P, L], F32, tag="xt")
        nc.vector.tensor_mul(Pt, Et, invC_b)
        nc.scalar.activation(
            Et, Pt, Sqrt, bias=sbias_sb[:, t : t + 1], scale=invR[:, t : t + 1]
        )
        nc.gpsimd.tensor_mul(Et[:, HALF:], Et[:, HALF:], cmask_h)
        nc.sync.dma_start(out=y[t * P : (t + 1) * P, :], in_=Et)


def _split_multi_waits(nc):
    """This walrus build's CoreV3 setupSyncWait rejects ANY instruction
    carrying more than one semaphore wait ("Too many sync wait commands");
    the ISA Events header has a single wait slot. Hoist extra waits onto
    preceding same-engine NoOps (sequential ge-waits on monotonic semaphores
    are equivalent to a combined wait). Apply only for the HW path — the
    synthetic NoOps lack the sim's sem bookkeeping and break CoreSim."""
    n = 0
    for fn in nc.m.functions:
        for bb in fn.blocks:
            out = []
            changed = False
            for inst in bb.instructions:
                si = inst.sync_info
                waits = list(si.on_wait) if (si and si.on_wait) else []
                if len(waits) > 1:
                    for w in waits[:-1]:
                        n += 1
                        out.append(
                            mybir.InstNoOp(
                                name=f"antsplitwait-{n}",
                                engine=inst.engine,
                                sync_info=mybir.SyncInfo(on_wait=[w], on_update=[]),
                            )
                        )
                    si.on_wait = waits[-1:]
                    changed = True
                out.append(inst)
            if changed:
                bb.instructions = out
    return nc


def build_nc(split_waits=True):
    nc = bass.Bass()
    x = nc.dram_tensor("x", [L, L], F32, kind="ExternalInput")
    cneg = nc.dram_tensor("cneg", [1, L], F32, kind="ExternalInput")
    cmask = nc.dram_tensor("cmask", [1, L], F32, kind="ExternalInput")
    auxT = nc.dram_tensor("auxT", [P, 4 * NT], F32, kind="ExternalInput")
    cdram = nc.dram_tensor("cscratch", [NT, P], F32, kind="Internal")
    crdram = nc.dram_tensor("crscratch", [1, L], F32, kind="Internal")
    y = nc.dram_tensor("y", [L, L], F32, kind="ExternalOutput")

    with tile.TileContext(nc) as tc, ExitStack() as ctx:
        _body(ctx, tc, x, cneg, cmask, auxT, cdram, crdram, y)
    if split_waits:
        _split_multi_waits(nc)
    return nc


def get_nc():
    if "nc" not in _CACHE:
        _CACHE["nc"] = build_nc()
    return _CACHE["nc"]


def make_in_maps(sim_matrix, lengths):
    sim_matrix = np.ascontiguousarray(np.asarray(sim_matrix, dtype=np.float32))
    lengths = np.asarray(lengths, dtype=np.int32)
    idx = np.arange(L)
    in_maps = []
    for c in range(sim_matrix.shape[0]):
        l1, l2 = int(lengths[c, 0]), int(lengths[c, 1])
        rv = idx < l1  # row valid
        cv = idx < l2  # col valid

        def tcol(vals):  # [2048] -> [128, 16] with element i at [i%128, i//128]
            return np.ascontiguousarray(
                np.asarray(vals, dtype=np.float32).reshape(NT, P).T
            )

        auxT = np.concatenate(
            [
                tcol(np.where(rv, -MSTAB, -MSTAB - NEGB)),  # rbias
                tcol(np.where(rv, EPS, 0.0)),  # sbias
                tcol(np.where(rv, 0.0, 1.0)),  # rfix
                tcol(np.where(cv, 0.0, 1.0)),  # cfix
            ],
            axis=1,
        )
        in_maps.append(
            {
                "x": sim_matrix[c],
                "cneg": np.where(cv, 0.0, -NEGB / 2).astype(np.float32)[None, :],
                "cmask": cv.astype(np.float32)[None, :],
                "auxT": np.ascontiguousarray(auxT),
            }
        )
    return in_maps


def run(sim_matrix, lengths, trace=False):
    nc = get_nc()
    in_maps = make_in_maps(sim_matrix, lengths)
    res = run_bass_kernel_spmd(nc, in_maps, list(range(B)), trace=trace)
    out = np.stack([res.results[c]["y"] for c in range(B)], axis=0)
    return out, res


def kernel(sim_matrix, lengths):
    out, _ = run(sim_matrix, lengths, trace=False)
    return out
